# revision 21
# baseline (speedup 1.0000x reference)
"""YOLO box-decode kernel for Trainium2 (Bass/Tile), 8-core data parallel.

Contract: kernel(**inputs) takes the FULL inputs from setup_inputs()
(x: [32,255,80,80] f32, anchors: [3,2] f32) and returns the full
(boxes [32,3,80,80,6] f32, mask [32,3,80,80] bool) like the reference.

Sharding: pure data parallel over the batch axis - 4 images per core,
no cross-core communication.

Per-core layout: 4 images x 3 anchors = 12 blocks, each a contiguous
[85, 6400] f32 region of DRAM. Three SBUF tiles of [128, 85, 200]: each
tile packs 4 blocks (partition groups of 32), so every DMA row is 200
contiguous f32 (800B) - above the 512B threshold where the DMA engines
run at full rate.

Compute per tile:
  ACT : sigmoid(ch0:2) + grid offsets, exp(ch2:4) * anchor dims
  DVE : score = cls * obj (in-place over cls, obj broadcast along ch)
        best  = reduce_max(score over ch)
        eq    = (score == best)            (u8)
        cand  = eq * (ch_index - 128)      (in-place over score)
        cls   = reduce_min(cand over ch) + 128
The eq/min construction reproduces jnp.argmax's first-occurrence
tie-breaking exactly; score/best/cls are bit-exact vs the f32 reference.
mask = best > 0.5 is derived on the host from best_score.
"""

import sys

for _p in ("/opt/trn_rl_repo", "/opt/pypackages"):
    if _p not in sys.path:
        sys.path.insert(0, _p)

import numpy as np

N, C, H, W = 32, 255, 80, 80
A = 3                 # anchors
V = 85                # values per anchor (5 + CLS)
CLS = 80
HW = H * W            # 6400
NCORES = 8
NSH = N // NCORES     # images per core
NBLK = NSH * A        # (n, a) blocks per core
TPB = 4               # blocks packed per SBUF tile (32 partitions each)
NT = NBLK // TPB      # SBUF tiles per core
J = HW // 32          # free-dim cells per partition (200)
BIG = 128.0           # argmax bias; any value > CLS with exact f32 ints
CONF_THR = 0.5


def _build(anchors: np.ndarray):
    import concourse.bass as bass
    import concourse.bacc as bacc
    import concourse.tile as tile
    from concourse import mybir

    f32 = mybir.dt.float32

    # Bacc (not plain Bass): its finalize() runs generate_event_semaphores,
    # which splits multi-semaphore waits - TRN2 allows 1 wait per instruction.
    nc = bacc.Bacc()
    x_h = nc.dram_tensor("x", [NSH, C, H, W], f32, kind="ExternalInput")
    out_h = nc.dram_tensor("out", [NSH, A, 6, HW], f32, kind="ExternalOutput")

    # Constant table, one row per partition:
    #   [0:200)   gx  grid x-offset per (partition, j) cell
    #   [200:400) gy  grid y-offset
    #   [400:403) anchor width for tile t (depends on p//32 -> block -> a)
    #   [403:406) anchor height for tile t
    #   [406:486) ch_index - BIG  (argmax payload)
    p32 = np.arange(128) % 32
    cell = p32[:, None] * J + np.arange(J)[None, :]
    gx = (cell % W).astype(np.float32)
    gy = (cell // W).astype(np.float32)
    anchw = np.zeros((128, NT), np.float32)
    anchh = np.zeros((128, NT), np.float32)
    for t in range(NT):
        for g in range(TPB):
            a = (t * TPB + g) % A
            anchw[g * 32:(g + 1) * 32, t] = anchors[a, 0]
            anchh[g * 32:(g + 1) * 32, t] = anchors[a, 1]
    iota = np.tile((np.arange(CLS) - BIG).astype(np.float32), (128, 1))
    cgrid_np = np.concatenate([gx, gy, anchw, anchh, iota], axis=1)
    cg_h = nc.inline_tensor(cgrid_np.astype(np.float32), name="cgrid")

    # whole shard as [NBLK*85, 6400]: 255 = 3*85, so the (n, a) blocks tile
    # the channel axis contiguously
    xv = x_h[:, :, :, :].rearrange("n c h w -> (n c) (h w)")
    ov = out_h[:, :, :, :].rearrange("n a k hw -> (n a k) hw")

    Sigmoid = mybir.ActivationFunctionType.Sigmoid
    Exp = mybir.ActivationFunctionType.Exp
    X = mybir.AxisListType.X
    op = mybir.AluOpType

    bf16 = mybir.dt.bfloat16

    with tile.TileContext(nc) as tc:
        with (
            tc.tile_pool(name="xsp", bufs=2) as xsp,
            tc.tile_pool(name="xcp", bufs=2) as xcp,
            tc.tile_pool(name="scrp", bufs=1) as scrp,
            tc.tile_pool(name="outp", bufs=2) as outp,
            tc.tile_pool(name="constp", bufs=1) as constp,
        ):
            cg = constp.tile([128, cgrid_np.shape[1]], f32)
            nc.gpsimd.dma_start(out=cg, in_=cg_h[:, :])
            gxy = cg[:, 0:400].rearrange("p (c j) -> p c j", c=2)
            iotb = constp.tile([128, CLS], bf16)
            nc.vector.tensor_copy(out=iotb, in_=cg[:, 406:406 + CLS])

            for t in range(NT):
                # xs: xy/wh/obj channels; xc: class channels, overwritten in
                # place by score = cls * obj (all ch-major, unit-stride)
                xs = xsp.tile([128, 5, J], f32)
                xc = xcp.tile([128, CLS, J], f32)
                bx6 = outp.tile([128, 6, J], f32)
                # scr: f32 max-tree scratch, then (bitcast) bf16 eq/cand
                scr = scrp.tile([128, 40, J], f32)
                scrb = (
                    scr[:, :, :]
                    .rearrange("p a (b j) -> p (a b) j", b=2)
                    .bitcast(bf16)
                )  # [128, CLS, J] bf16 view of the same bytes

                for g in range(TPB):
                    b = t * TPB + g
                    ps = slice(g * 32, (g + 1) * 32)
                    nc.sync.dma_start(
                        out=xs[ps, :, :],
                        in_=xv[b * V:b * V + 5, :].rearrange(
                            "c (p j) -> p c j", j=J
                        ),
                    )
                    nc.sync.dma_start(
                        out=xc[ps, :, :],
                        in_=xv[b * V + 5:(b + 1) * V, :].rearrange(
                            "c (p j) -> p c j", j=J
                        ),
                    )
                # score = cls * obj on pool, in place (xc double-buffered, so
                # this overlaps the previous tile's DVE chain)
                nc.gpsimd.tensor_tensor(
                    out=xc,
                    in0=xc,
                    in1=xs[:, 4:5, :].broadcast_to((128, CLS, J)),
                    op=op.mult,
                )

                # centers/sizes on ACT (batched by function to avoid
                # activation-table reloads), grid add on DVE
                nc.scalar.activation(
                    out=bx6[:, 0:2, :], in_=xs[:, 0:2, :], func=Sigmoid
                )
                nc.scalar.activation(
                    out=bx6[:, 2:4, :], in_=xs[:, 2:4, :], func=Exp
                )
                nc.vector.tensor_add(out=bx6[:, 0:2, :], in0=bx6[:, 0:2, :], in1=gxy)
                nc.scalar.mul(
                    out=bx6[:, 2, :], in_=bx6[:, 2, :], mul=cg[:, 400 + t:401 + t]
                )
                nc.scalar.mul(
                    out=bx6[:, 3, :], in_=bx6[:, 3, :], mul=cg[:, 403 + t:404 + t]
                )

                # best = max over ch: pairwise max tree, all unit-stride
                # (DVE; the pool ucode rejects max)
                nc.vector.tensor_tensor(
                    out=scr, in0=xc[:, 0:40, :], in1=xc[:, 40:80, :], op=op.max
                )
                for w in (20, 10, 5):
                    nc.vector.tensor_tensor(
                        out=scr[:, 0:w, :], in0=scr[:, 0:w, :],
                        in1=scr[:, w:2 * w, :], op=op.max,
                    )
                nc.vector.tensor_tensor(
                    out=scr[:, 0:2, :], in0=scr[:, 0:2, :], in1=scr[:, 2:4, :],
                    op=op.max,
                )
                nc.vector.tensor_tensor(
                    out=scr[:, 0, :], in0=scr[:, 0, :], in1=scr[:, 1, :],
                    op=op.max,
                )
                nc.vector.tensor_tensor(
                    out=bx6[:, 4, :], in0=scr[:, 0, :], in1=scr[:, 4, :],
                    op=op.max,
                )

                # eq = (score == best) -> bf16, unit-stride
                nc.vector.tensor_tensor(
                    out=scrb,
                    in0=xc,
                    in1=bx6[:, 4:5, :].broadcast_to((128, CLS, J)),
                    op=op.is_equal,
                )
                # cand = eq * (ch - BIG), in place, all-bf16
                nc.vector.tensor_tensor(
                    out=scrb,
                    in0=scrb,
                    in1=iotb.unsqueeze(2).broadcast_to((128, CLS, J)),
                    op=op.mult,
                )
                # best_cls = min(cand) + BIG via bf16 pairwise min tree
                # (2x DVE mode; a tensor_reduce here measures 2x slower)
                nc.vector.tensor_tensor(
                    out=scrb[:, 0:40, :], in0=scrb[:, 0:40, :],
                    in1=scrb[:, 40:80, :], op=op.min,
                )
                for w in (20, 10, 5):
                    nc.vector.tensor_tensor(
                        out=scrb[:, 0:w, :], in0=scrb[:, 0:w, :],
                        in1=scrb[:, w:2 * w, :], op=op.min,
                    )
                nc.vector.tensor_tensor(
                    out=scrb[:, 0:2, :], in0=scrb[:, 0:2, :],
                    in1=scrb[:, 2:4, :], op=op.min,
                )
                nc.vector.tensor_tensor(
                    out=scrb[:, 0, :], in0=scrb[:, 0, :], in1=scrb[:, 1, :],
                    op=op.min,
                )
                cmin = outp.tile([128, J], bf16, tag="cmin")
                nc.vector.tensor_tensor(
                    out=cmin, in0=scrb[:, 0, :], in1=scrb[:, 4, :], op=op.min
                )
                nc.vector.tensor_scalar_add(
                    out=bx6[:, 5, :], in0=cmin, scalar1=BIG
                )

                for g in range(TPB):
                    b = t * TPB + g
                    dst = ov[b * 6:(b + 1) * 6, :].rearrange(
                        "k (p j) -> p k j", j=J
                    )
                    # ACT's HWDGE queue: separate from the load queue (no
                    # head-of-line blocking) and off the pool's instruction
                    # stream (which would stall the next tile's score mult)
                    nc.scalar.dma_start(
                        out=dst, in_=bx6[g * 32:(g + 1) * 32, :, :]
                    )

    return nc


def _assemble(core_outs):
    out = np.concatenate(core_outs, axis=0)           # [N, A, 6, HW]
    nb = out.shape[0]
    boxes = np.transpose(out, (0, 1, 3, 2)).reshape(nb, A, H, W, 6)
    boxes = np.ascontiguousarray(boxes, dtype=np.float32)
    mask = boxes[..., 4] > CONF_THR
    return boxes, mask


def kernel(**inputs):
    x = np.ascontiguousarray(inputs["x"], dtype=np.float32)
    anchors = np.asarray(inputs["anchors"], dtype=np.float32)
    assert x.shape == (N, C, H, W), x.shape

    from concourse.bass_utils import run_bass_kernel_spmd

    nc = _build(anchors)
    nc.finalize()  # Bacc lowering (reg alloc, wait splitting) before PJRT
    in_maps = [{"x": x[k * NSH:(k + 1) * NSH]} for k in range(NCORES)]
    res = run_bass_kernel_spmd(nc, in_maps, list(range(NCORES))).results
    return _assemble([res[k]["out"] for k in range(NCORES)])


# revision 22
# speedup vs baseline: 1.1632x; 1.1632x over previous
"""YOLO box-decode kernel for Trainium2 (Bass/Tile), 8-core data parallel.

Contract: kernel(**inputs) takes the FULL inputs from setup_inputs()
(x: [32,255,80,80] f32, anchors: [3,2] f32) and returns the full
(boxes [32,3,80,80,6] f32, mask [32,3,80,80] bool) like the reference.

Sharding: pure data parallel over the batch axis - 4 images per core,
no cross-core communication.

Per-core layout: 4 images x 3 anchors = 12 blocks, each a contiguous
[85, 6400] f32 region of DRAM. Three SBUF tiles of [128, 85, 200]: each
tile packs 4 blocks (partition groups of 32), so every DMA row is 200
contiguous f32 (800B) - above the 512B threshold where the DMA engines
run at full rate.

Compute per tile:
  ACT : sigmoid(ch0:2) + grid offsets, exp(ch2:4) * anchor dims
  DVE : score = cls * obj (in-place over cls, obj broadcast along ch)
        best  = reduce_max(score over ch)
        eq    = (score == best)            (u8)
        cand  = eq * (ch_index - 128)      (in-place over score)
        cls   = reduce_min(cand over ch) + 128
The eq/min construction reproduces jnp.argmax's first-occurrence
tie-breaking exactly; score/best/cls are bit-exact vs the f32 reference.
mask = best > 0.5 is derived on the host from best_score.
"""

import sys

for _p in ("/opt/trn_rl_repo", "/opt/pypackages"):
    if _p not in sys.path:
        sys.path.insert(0, _p)

import numpy as np

N, C, H, W = 32, 255, 80, 80
A = 3                 # anchors
V = 85                # values per anchor (5 + CLS)
CLS = 80
HW = H * W            # 6400
NCORES = 8
NSH = N // NCORES     # images per core
NBLK = NSH * A        # (n, a) blocks per core
TPB = 4               # blocks packed per SBUF tile (32 partitions each)
NT = NBLK // TPB      # SBUF tiles per core
J = HW // 32          # free-dim cells per partition (200)
BIG = 128.0           # argmax bias; any value > CLS with exact f32 ints
CONF_THR = 0.5


def _build(anchors: np.ndarray):
    import concourse.bass as bass
    import concourse.bacc as bacc
    import concourse.tile as tile
    from concourse import mybir

    f32 = mybir.dt.float32

    # Bacc (not plain Bass): its finalize() runs generate_event_semaphores,
    # which splits multi-semaphore waits - TRN2 allows 1 wait per instruction.
    nc = bacc.Bacc()
    x_h = nc.dram_tensor("x", [NSH, C, H, W], f32, kind="ExternalInput")
    out_h = nc.dram_tensor("out", [NSH, A, 6, HW], f32, kind="ExternalOutput")

    # Constant table, one row per partition:
    #   [0:200)   gx  grid x-offset per (partition, j) cell
    #   [200:400) gy  grid y-offset
    #   [400:403) anchor width for tile t (depends on p//32 -> block -> a)
    #   [403:406) anchor height for tile t
    #   [406:486) ch_index - BIG  (argmax payload)
    p32 = np.arange(128) % 32
    cell = p32[:, None] * J + np.arange(J)[None, :]
    gx = (cell % W).astype(np.float32)
    gy = (cell // W).astype(np.float32)
    anchw = np.zeros((128, NT), np.float32)
    anchh = np.zeros((128, NT), np.float32)
    for t in range(NT):
        for g in range(TPB):
            a = (t * TPB + g) % A
            anchw[g * 32:(g + 1) * 32, t] = anchors[a, 0]
            anchh[g * 32:(g + 1) * 32, t] = anchors[a, 1]
    iota = np.tile((np.arange(CLS) - BIG).astype(np.float32), (128, 1))
    cgrid_np = np.concatenate([gx, gy, anchw, anchh, iota], axis=1)
    cg_h = nc.inline_tensor(cgrid_np.astype(np.float32), name="cgrid")

    # whole shard as [NBLK*85, 6400]: 255 = 3*85, so the (n, a) blocks tile
    # the channel axis contiguously
    xv = x_h[:, :, :, :].rearrange("n c h w -> (n c) (h w)")
    ov = out_h[:, :, :, :].rearrange("n a k hw -> (n a k) hw")

    Sigmoid = mybir.ActivationFunctionType.Sigmoid
    Exp = mybir.ActivationFunctionType.Exp
    X = mybir.AxisListType.X
    op = mybir.AluOpType

    bf16 = mybir.dt.bfloat16

    with tile.TileContext(nc) as tc:
        with (
            tc.tile_pool(name="xsp", bufs=2) as xsp,
            tc.tile_pool(name="xcp", bufs=2) as xcp,
            tc.tile_pool(name="scrp", bufs=1) as scrp,
            tc.tile_pool(name="outp", bufs=2) as outp,
            tc.tile_pool(name="constp", bufs=1) as constp,
        ):
            cg = constp.tile([128, cgrid_np.shape[1]], f32)
            nc.gpsimd.dma_start(out=cg, in_=cg_h[:, :])
            gxy = cg[:, 0:400].rearrange("p (c j) -> p c j", c=2)
            iotb = constp.tile([128, CLS], bf16)
            nc.vector.tensor_copy(out=iotb, in_=cg[:, 406:406 + CLS])

            for t in range(NT):
                # xs: xy/wh/obj channels; xc: class channels, overwritten in
                # place by score = cls * obj (all ch-major, unit-stride)
                xs = xsp.tile([128, 5, J], f32)
                xc = xcp.tile([128, CLS, J], f32)
                bx6 = outp.tile([128, 6, J], f32)
                # scr: f32 max-tree scratch, then (bitcast) bf16 eq/cand
                scr = scrp.tile([128, 40, J], f32)
                scrb = (
                    scr[:, :, :]
                    .rearrange("p a (b j) -> p (a b) j", b=2)
                    .bitcast(bf16)
                )  # [128, CLS, J] bf16 view of the same bytes

                for g in range(TPB):
                    b = t * TPB + g
                    ps = slice(g * 32, (g + 1) * 32)
                    nc.sync.dma_start(
                        out=xs[ps, :, :],
                        in_=xv[b * V:b * V + 5, :].rearrange(
                            "c (p j) -> p c j", j=J
                        ),
                    )
                    nc.sync.dma_start(
                        out=xc[ps, :, :],
                        in_=xv[b * V + 5:(b + 1) * V, :].rearrange(
                            "c (p j) -> p c j", j=J
                        ),
                    )
                # score = cls * obj, in place. On DVE: pool and DVE streaming
                # SBUF concurrently starve each other (measured 8-16x DVE
                # slowdown), so a pool offload is a net loss.
                nc.vector.tensor_tensor(
                    out=xc,
                    in0=xc,
                    in1=xs[:, 4:5, :].broadcast_to((128, CLS, J)),
                    op=op.mult,
                )

                # centers/sizes on ACT (batched by function to avoid
                # activation-table reloads), grid add on DVE
                nc.scalar.activation(
                    out=bx6[:, 0:2, :], in_=xs[:, 0:2, :], func=Sigmoid
                )
                nc.scalar.activation(
                    out=bx6[:, 2:4, :], in_=xs[:, 2:4, :], func=Exp
                )
                nc.vector.tensor_add(out=bx6[:, 0:2, :], in0=bx6[:, 0:2, :], in1=gxy)
                nc.scalar.mul(
                    out=bx6[:, 2, :], in_=bx6[:, 2, :], mul=cg[:, 400 + t:401 + t]
                )
                nc.scalar.mul(
                    out=bx6[:, 3, :], in_=bx6[:, 3, :], mul=cg[:, 403 + t:404 + t]
                )

                # best = max over ch: pairwise max tree, all unit-stride
                # (DVE; the pool ucode rejects max)
                nc.vector.tensor_tensor(
                    out=scr, in0=xc[:, 0:40, :], in1=xc[:, 40:80, :], op=op.max
                )
                for w in (20, 10, 5):
                    nc.vector.tensor_tensor(
                        out=scr[:, 0:w, :], in0=scr[:, 0:w, :],
                        in1=scr[:, w:2 * w, :], op=op.max,
                    )
                nc.vector.tensor_tensor(
                    out=scr[:, 0:2, :], in0=scr[:, 0:2, :], in1=scr[:, 2:4, :],
                    op=op.max,
                )
                nc.vector.tensor_tensor(
                    out=scr[:, 0, :], in0=scr[:, 0, :], in1=scr[:, 1, :],
                    op=op.max,
                )
                nc.vector.tensor_tensor(
                    out=bx6[:, 4, :], in0=scr[:, 0, :], in1=scr[:, 4, :],
                    op=op.max,
                )

                # eq = (score == best) -> bf16, unit-stride
                nc.vector.tensor_tensor(
                    out=scrb,
                    in0=xc,
                    in1=bx6[:, 4:5, :].broadcast_to((128, CLS, J)),
                    op=op.is_equal,
                )
                # cand = eq * (ch - BIG), in place, all-bf16
                nc.vector.tensor_tensor(
                    out=scrb,
                    in0=scrb,
                    in1=iotb.unsqueeze(2).broadcast_to((128, CLS, J)),
                    op=op.mult,
                )
                # best_cls = min(cand) + BIG via bf16 pairwise min tree
                # (2x DVE mode; a tensor_reduce here measures 2x slower)
                nc.vector.tensor_tensor(
                    out=scrb[:, 0:40, :], in0=scrb[:, 0:40, :],
                    in1=scrb[:, 40:80, :], op=op.min,
                )
                for w in (20, 10, 5):
                    nc.vector.tensor_tensor(
                        out=scrb[:, 0:w, :], in0=scrb[:, 0:w, :],
                        in1=scrb[:, w:2 * w, :], op=op.min,
                    )
                nc.vector.tensor_tensor(
                    out=scrb[:, 0:2, :], in0=scrb[:, 0:2, :],
                    in1=scrb[:, 2:4, :], op=op.min,
                )
                nc.vector.tensor_tensor(
                    out=scrb[:, 0, :], in0=scrb[:, 0, :], in1=scrb[:, 1, :],
                    op=op.min,
                )
                cmin = outp.tile([128, J], bf16, tag="cmin")
                nc.vector.tensor_tensor(
                    out=cmin, in0=scrb[:, 0, :], in1=scrb[:, 4, :], op=op.min
                )
                nc.vector.tensor_scalar_add(
                    out=bx6[:, 5, :], in0=cmin, scalar1=BIG
                )

                for g in range(TPB):
                    b = t * TPB + g
                    dst = ov[b * 6:(b + 1) * 6, :].rearrange(
                        "k (p j) -> p k j", j=J
                    )
                    # ACT's HWDGE queue: separate from the load queue (no
                    # head-of-line blocking) and off the pool's instruction
                    # stream (which would stall the next tile's score mult)
                    nc.scalar.dma_start(
                        out=dst, in_=bx6[g * 32:(g + 1) * 32, :, :]
                    )

    return nc


def _assemble(core_outs):
    out = np.concatenate(core_outs, axis=0)           # [N, A, 6, HW]
    nb = out.shape[0]
    boxes = np.transpose(out, (0, 1, 3, 2)).reshape(nb, A, H, W, 6)
    boxes = np.ascontiguousarray(boxes, dtype=np.float32)
    mask = boxes[..., 4] > CONF_THR
    return boxes, mask


def kernel(**inputs):
    x = np.ascontiguousarray(inputs["x"], dtype=np.float32)
    anchors = np.asarray(inputs["anchors"], dtype=np.float32)
    assert x.shape == (N, C, H, W), x.shape

    from concourse.bass_utils import run_bass_kernel_spmd

    nc = _build(anchors)
    nc.finalize()  # Bacc lowering (reg alloc, wait splitting) before PJRT
    in_maps = [{"x": x[k * NSH:(k + 1) * NSH]} for k in range(NCORES)]
    res = run_bass_kernel_spmd(nc, in_maps, list(range(NCORES))).results
    return _assemble([res[k]["out"] for k in range(NCORES)])


# revision 23
# speedup vs baseline: 1.2667x; 1.0890x over previous
"""YOLO box-decode kernel for Trainium2 (Bass/Tile), 8-core data parallel.

Contract: kernel(**inputs) takes the FULL inputs from setup_inputs()
(x: [32,255,80,80] f32, anchors: [3,2] f32) and returns the full
(boxes [32,3,80,80,6] f32, mask [32,3,80,80] bool) like the reference.

Sharding: pure data parallel over the batch axis - 4 images per core,
no cross-core communication.

Per-core layout: 4 images x 3 anchors = 12 blocks, each a contiguous
[85, 6400] f32 region of DRAM. Three SBUF tiles of [128, 85, 200]: each
tile packs 4 blocks (partition groups of 32), so every DMA row is 200
contiguous f32 (800B) - above the 512B threshold where the DMA engines
run at full rate.

Compute per tile:
  ACT : sigmoid(ch0:2) + grid offsets, exp(ch2:4) * anchor dims
  DVE : score = cls * obj (in-place over cls, obj broadcast along ch)
        best  = reduce_max(score over ch)
        eq    = (score == best)            (u8)
        cand  = eq * (ch_index - 128)      (in-place over score)
        cls   = reduce_min(cand over ch) + 128
The eq/min construction reproduces jnp.argmax's first-occurrence
tie-breaking exactly; score/best/cls are bit-exact vs the f32 reference.
mask = best > 0.5 is derived on the host from best_score.
"""

import sys

for _p in ("/opt/trn_rl_repo", "/opt/pypackages"):
    if _p not in sys.path:
        sys.path.insert(0, _p)

import numpy as np

N, C, H, W = 32, 255, 80, 80
A = 3                 # anchors
V = 85                # values per anchor (5 + CLS)
CLS = 80
HW = H * W            # 6400
NCORES = 8
NSH = N // NCORES     # images per core
NBLK = NSH * A        # (n, a) blocks per core
TPB = 4               # blocks packed per SBUF tile (32 partitions each)
NT = NBLK // TPB      # SBUF tiles per core
J = HW // 32          # free-dim cells per partition (200)
BIG = 128.0           # argmax bias; any value > CLS with exact f32 ints
CONF_THR = 0.5


def _build(anchors: np.ndarray):
    import concourse.bass as bass
    import concourse.bacc as bacc
    import concourse.tile as tile
    from concourse import mybir

    f32 = mybir.dt.float32

    # Bacc (not plain Bass): its finalize() runs generate_event_semaphores,
    # which splits multi-semaphore waits - TRN2 allows 1 wait per instruction.
    nc = bacc.Bacc()
    x_h = nc.dram_tensor("x", [NSH, C, H, W], f32, kind="ExternalInput")
    out_h = nc.dram_tensor("out", [NSH, A, 6, HW], f32, kind="ExternalOutput")

    # Constant table, one row per partition:
    #   [0:200)   gx  grid x-offset per (partition, j) cell
    #   [200:400) gy  grid y-offset
    #   [400:403) anchor width for tile t (depends on p//32 -> block -> a)
    #   [403:406) anchor height for tile t
    #   [406:486) ch_index - BIG  (argmax payload)
    p32 = np.arange(128) % 32
    cell = p32[:, None] * J + np.arange(J)[None, :]
    gx = (cell % W).astype(np.float32)
    gy = (cell // W).astype(np.float32)
    anchw = np.zeros((128, NT), np.float32)
    anchh = np.zeros((128, NT), np.float32)
    for t in range(NT):
        for g in range(TPB):
            a = (t * TPB + g) % A
            anchw[g * 32:(g + 1) * 32, t] = anchors[a, 0]
            anchh[g * 32:(g + 1) * 32, t] = anchors[a, 1]
    iota = np.tile((np.arange(CLS) - BIG).astype(np.float32), (128, 1))
    cgrid_np = np.concatenate([gx, gy, anchw, anchh, iota], axis=1)
    cg_h = nc.inline_tensor(cgrid_np.astype(np.float32), name="cgrid")

    # whole shard as [NBLK*85, 6400]: 255 = 3*85, so the (n, a) blocks tile
    # the channel axis contiguously
    xv = x_h[:, :, :, :].rearrange("n c h w -> (n c) (h w)")
    ov = out_h[:, :, :, :].rearrange("n a k hw -> (n a k) hw")

    Sigmoid = mybir.ActivationFunctionType.Sigmoid
    Exp = mybir.ActivationFunctionType.Exp
    X = mybir.AxisListType.X
    op = mybir.AluOpType

    bf16 = mybir.dt.bfloat16

    with tile.TileContext(nc) as tc:
        with (
            tc.tile_pool(name="xsp", bufs=2) as xsp,
            tc.tile_pool(name="xcp", bufs=2) as xcp,
            tc.tile_pool(name="scrp", bufs=1) as scrp,
            tc.tile_pool(name="outp", bufs=2) as outp,
            tc.tile_pool(name="constp", bufs=1) as constp,
        ):
            cg = constp.tile([128, cgrid_np.shape[1]], f32)
            nc.gpsimd.dma_start(out=cg, in_=cg_h[:, :])
            gxy = cg[:, 0:400].rearrange("p (c j) -> p c j", c=2)
            iotb = constp.tile([128, CLS], bf16)
            nc.vector.tensor_copy(out=iotb, in_=cg[:, 406:406 + CLS])

            for t in range(NT):
                # xs: xy/wh/obj channels; xc: class channels, overwritten in
                # place by score = cls * obj (all ch-major, unit-stride)
                xs = xsp.tile([128, 5, J], f32)
                xc = xcp.tile([128, CLS, J], f32)
                bx6 = outp.tile([128, 6, J], f32)
                # scr: f32 max-tree scratch, then (bitcast) bf16 eq/cand
                scr = scrp.tile([128, 40, J], f32)
                scrb = (
                    scr[:, :, :]
                    .rearrange("p a (b j) -> p (a b) j", b=2)
                    .bitcast(bf16)
                )  # [128, CLS, J] bf16 view of the same bytes

                for g in range(TPB):
                    b = t * TPB + g
                    ps = slice(g * 32, (g + 1) * 32)
                    # split the big loads across both HWDGE queues (SP, ACT)
                    # so the per-queue serial transfer chain halves
                    ldq = nc.sync if g < 2 else nc.scalar
                    ldq.dma_start(
                        out=xs[ps, :, :],
                        in_=xv[b * V:b * V + 5, :].rearrange(
                            "c (p j) -> p c j", j=J
                        ),
                    )
                    ldq.dma_start(
                        out=xc[ps, :, :],
                        in_=xv[b * V + 5:(b + 1) * V, :].rearrange(
                            "c (p j) -> p c j", j=J
                        ),
                    )
                # score = cls * obj, in place. On DVE: pool and DVE streaming
                # SBUF concurrently starve each other (measured 8-16x DVE
                # slowdown), so a pool offload is a net loss.
                nc.vector.tensor_tensor(
                    out=xc,
                    in0=xc,
                    in1=xs[:, 4:5, :].broadcast_to((128, CLS, J)),
                    op=op.mult,
                )

                # centers/sizes on ACT (batched by function to avoid
                # activation-table reloads), grid add on DVE
                nc.scalar.activation(
                    out=bx6[:, 0:2, :], in_=xs[:, 0:2, :], func=Sigmoid
                )
                nc.scalar.activation(
                    out=bx6[:, 2:4, :], in_=xs[:, 2:4, :], func=Exp
                )
                nc.vector.tensor_add(out=bx6[:, 0:2, :], in0=bx6[:, 0:2, :], in1=gxy)
                nc.scalar.mul(
                    out=bx6[:, 2, :], in_=bx6[:, 2, :], mul=cg[:, 400 + t:401 + t]
                )
                nc.scalar.mul(
                    out=bx6[:, 3, :], in_=bx6[:, 3, :], mul=cg[:, 403 + t:404 + t]
                )

                # best = max over ch: pairwise max tree, all unit-stride
                # (DVE; the pool ucode rejects max)
                nc.vector.tensor_tensor(
                    out=scr, in0=xc[:, 0:40, :], in1=xc[:, 40:80, :], op=op.max
                )
                for w in (20, 10, 5):
                    nc.vector.tensor_tensor(
                        out=scr[:, 0:w, :], in0=scr[:, 0:w, :],
                        in1=scr[:, w:2 * w, :], op=op.max,
                    )
                nc.vector.tensor_tensor(
                    out=scr[:, 0:2, :], in0=scr[:, 0:2, :], in1=scr[:, 2:4, :],
                    op=op.max,
                )
                nc.vector.tensor_tensor(
                    out=scr[:, 0, :], in0=scr[:, 0, :], in1=scr[:, 1, :],
                    op=op.max,
                )
                nc.vector.tensor_tensor(
                    out=bx6[:, 4, :], in0=scr[:, 0, :], in1=scr[:, 4, :],
                    op=op.max,
                )

                # eq = (score == best) -> bf16, unit-stride
                nc.vector.tensor_tensor(
                    out=scrb,
                    in0=xc,
                    in1=bx6[:, 4:5, :].broadcast_to((128, CLS, J)),
                    op=op.is_equal,
                )
                # cand = eq * (ch - BIG), in place, all-bf16
                nc.vector.tensor_tensor(
                    out=scrb,
                    in0=scrb,
                    in1=iotb.unsqueeze(2).broadcast_to((128, CLS, J)),
                    op=op.mult,
                )
                # best_cls = min(cand) + BIG via bf16 pairwise min tree
                # (2x DVE mode; a tensor_reduce here measures 2x slower)
                nc.vector.tensor_tensor(
                    out=scrb[:, 0:40, :], in0=scrb[:, 0:40, :],
                    in1=scrb[:, 40:80, :], op=op.min,
                )
                for w in (20, 10, 5):
                    nc.vector.tensor_tensor(
                        out=scrb[:, 0:w, :], in0=scrb[:, 0:w, :],
                        in1=scrb[:, w:2 * w, :], op=op.min,
                    )
                nc.vector.tensor_tensor(
                    out=scrb[:, 0:2, :], in0=scrb[:, 0:2, :],
                    in1=scrb[:, 2:4, :], op=op.min,
                )
                nc.vector.tensor_tensor(
                    out=scrb[:, 0, :], in0=scrb[:, 0, :], in1=scrb[:, 1, :],
                    op=op.min,
                )
                cmin = outp.tile([128, J], bf16, tag="cmin")
                nc.vector.tensor_tensor(
                    out=cmin, in0=scrb[:, 0, :], in1=scrb[:, 4, :], op=op.min
                )
                nc.vector.tensor_scalar_add(
                    out=bx6[:, 5, :], in0=cmin, scalar1=BIG
                )

                for g in range(TPB):
                    b = t * TPB + g
                    dst = ov[b * 6:(b + 1) * 6, :].rearrange(
                        "k (p j) -> p k j", j=J
                    )
                    # SWDGE: the pool engine is otherwise idle, so output
                    # DMA dispatch never blocks a load queue or compute
                    nc.gpsimd.dma_start(
                        out=dst, in_=bx6[g * 32:(g + 1) * 32, :, :]
                    )

    return nc


def _assemble(core_outs):
    out = np.concatenate(core_outs, axis=0)           # [N, A, 6, HW]
    nb = out.shape[0]
    boxes = np.transpose(out, (0, 1, 3, 2)).reshape(nb, A, H, W, 6)
    boxes = np.ascontiguousarray(boxes, dtype=np.float32)
    mask = boxes[..., 4] > CONF_THR
    return boxes, mask


def kernel(**inputs):
    x = np.ascontiguousarray(inputs["x"], dtype=np.float32)
    anchors = np.asarray(inputs["anchors"], dtype=np.float32)
    assert x.shape == (N, C, H, W), x.shape

    from concourse.bass_utils import run_bass_kernel_spmd

    nc = _build(anchors)
    nc.finalize()  # Bacc lowering (reg alloc, wait splitting) before PJRT
    in_maps = [{"x": x[k * NSH:(k + 1) * NSH]} for k in range(NCORES)]
    res = run_bass_kernel_spmd(nc, in_maps, list(range(NCORES))).results
    return _assemble([res[k]["out"] for k in range(NCORES)])


# revision 25
# speedup vs baseline: 1.3084x; 1.0329x over previous
"""YOLO box-decode kernel for Trainium2 (Bass/Tile), 8-core data parallel.

Contract: kernel(**inputs) takes the FULL inputs from setup_inputs()
(x: [32,255,80,80] f32, anchors: [3,2] f32) and returns the full
(boxes [32,3,80,80,6] f32, mask [32,3,80,80] bool) like the reference.

Sharding: pure data parallel over the batch axis - 4 images per core,
no cross-core communication.

Per-core layout: 4 images x 3 anchors = 12 blocks, each a contiguous
[85, 6400] f32 region of DRAM. Three rounds of SBUF tiles packing 4
blocks each (partition groups of 32), so every DMA row is 200
contiguous f32 (800B) - above the 512B threshold where the DMA engines
run at full rate. Loads are split across both HWDGE queues (SP + ACT);
output DMAs go through the idle pool's software DGE.

Compute per tile (ch-major, every op unit-stride on the inner axis):
  ACT : sigmoid(tx, ty), exp(tw, th) * anchor dims (per-partition scale)
  DVE : score = cls * obj        (in place over cls, obj broadcast)
        best  = pairwise max tree over ch (faster than strided reduce)
        eq    = (score == best)  -> bf16 into the max-tree scratch
        cand  = eq * (ch - 128)  (in place, bf16 2x mode)
        cls   = pairwise bf16 min tree over cand + 128
The eq/min construction reproduces jnp.argmax's first-occurrence
tie-breaking exactly; score/best/cls are bit-exact vs the f32 reference.
mask = best > 0.5 is derived on the host from best_score.
All big elementwise work stays on DVE: GPSIMD's tensor_tensor ucode only
supports basic arithmetic, and pool/DVE concurrent SBUF streaming
starves both engines (measured), so offloads there lose.
"""

import sys

for _p in ("/opt/trn_rl_repo", "/opt/pypackages"):
    if _p not in sys.path:
        sys.path.insert(0, _p)

import numpy as np

N, C, H, W = 32, 255, 80, 80
A = 3                 # anchors
V = 85                # values per anchor (5 + CLS)
CLS = 80
HW = H * W            # 6400
NCORES = 8
NSH = N // NCORES     # images per core
NBLK = NSH * A        # (n, a) blocks per core
TPB = 4               # blocks packed per SBUF tile (32 partitions each)
NT = NBLK // TPB      # SBUF tiles per core
J = HW // 32          # free-dim cells per partition (200)
BIG = 128.0           # argmax bias; any value > CLS with exact f32 ints
CONF_THR = 0.5


def _build(anchors: np.ndarray):
    import concourse.bass as bass
    import concourse.bacc as bacc
    import concourse.tile as tile
    from concourse import mybir

    f32 = mybir.dt.float32

    # Bacc (not plain Bass): its finalize() runs generate_event_semaphores,
    # which splits multi-semaphore waits - TRN2 allows 1 wait per instruction.
    nc = bacc.Bacc()
    x_h = nc.dram_tensor("x", [NSH, C, H, W], f32, kind="ExternalInput")
    out_h = nc.dram_tensor("out", [NSH, A, 6, HW], f32, kind="ExternalOutput")

    # Constant table, one row per partition:
    #   [0:200)   gx  grid x-offset per (partition, j) cell
    #   [200:400) gy  grid y-offset
    #   [400:403) anchor width for tile t (depends on p//32 -> block -> a)
    #   [403:406) anchor height for tile t
    #   [406:486) ch_index - BIG  (argmax payload)
    p32 = np.arange(128) % 32
    cell = p32[:, None] * J + np.arange(J)[None, :]
    gx = (cell % W).astype(np.float32)
    gy = (cell // W).astype(np.float32)
    anchw = np.zeros((128, NT), np.float32)
    anchh = np.zeros((128, NT), np.float32)
    for t in range(NT):
        for g in range(TPB):
            a = (t * TPB + g) % A
            anchw[g * 32:(g + 1) * 32, t] = anchors[a, 0]
            anchh[g * 32:(g + 1) * 32, t] = anchors[a, 1]
    iota = np.tile((np.arange(CLS) - BIG).astype(np.float32), (128, 1))
    cgrid_np = np.concatenate([gx, gy, anchw, anchh, iota], axis=1)
    cg_h = nc.inline_tensor(cgrid_np.astype(np.float32), name="cgrid")

    # whole shard as [NBLK*85, 6400]: 255 = 3*85, so the (n, a) blocks tile
    # the channel axis contiguously
    xv = x_h[:, :, :, :].rearrange("n c h w -> (n c) (h w)")
    ov = out_h[:, :, :, :].rearrange("n a k hw -> (n a k) hw")

    Sigmoid = mybir.ActivationFunctionType.Sigmoid
    Exp = mybir.ActivationFunctionType.Exp
    X = mybir.AxisListType.X
    op = mybir.AluOpType

    bf16 = mybir.dt.bfloat16

    with tile.TileContext(nc) as tc:
        with (
            tc.tile_pool(name="xsp", bufs=2) as xsp,
            tc.tile_pool(name="xcp", bufs=2) as xcp,
            tc.tile_pool(name="scrp", bufs=1) as scrp,
            tc.tile_pool(name="outp", bufs=2) as outp,
            tc.tile_pool(name="constp", bufs=1) as constp,
        ):
            cg = constp.tile([128, cgrid_np.shape[1]], f32)
            nc.gpsimd.dma_start(out=cg, in_=cg_h[:, :])
            gxy = cg[:, 0:400].rearrange("p (c j) -> p c j", c=2)
            iotb = constp.tile([128, CLS], bf16)
            nc.vector.tensor_copy(out=iotb, in_=cg[:, 406:406 + CLS])

            for t in range(NT):
                # xs: xy/wh/obj channels; xc: class channels, overwritten in
                # place by score = cls * obj (all ch-major, unit-stride)
                xs = xsp.tile([128, 5, J], f32)
                xc = xcp.tile([128, CLS, J], f32)
                bx6 = outp.tile([128, 6, J], f32)
                # scr: f32 max-tree scratch, then (bitcast) bf16 eq/cand
                scr = scrp.tile([128, 40, J], f32)
                scrb = (
                    scr[:, :, :]
                    .rearrange("p a (b j) -> p (a b) j", b=2)
                    .bitcast(bf16)
                )  # [128, CLS, J] bf16 view of the same bytes

                # loads split across both HWDGE queues (SP, ACT) and into
                # class halves: the lo-half score multiply starts after only
                # half the transfers, hiding load latency at tile 0
                for g in range(TPB):
                    b = t * TPB + g
                    ps = slice(g * 32, (g + 1) * 32)
                    ldq = nc.sync if g < 2 else nc.scalar
                    ldq.dma_start(
                        out=xs[ps, :, :],
                        in_=xv[b * V:b * V + 5, :].rearrange(
                            "c (p j) -> p c j", j=J
                        ),
                    )
                    ldq.dma_start(
                        out=xc[ps, 0:40, :],
                        in_=xv[b * V + 5:b * V + 45, :].rearrange(
                            "c (p j) -> p c j", j=J
                        ),
                    )
                for g in range(TPB):
                    b = t * TPB + g
                    ps = slice(g * 32, (g + 1) * 32)
                    ldq = nc.sync if g < 2 else nc.scalar
                    ldq.dma_start(
                        out=xc[ps, 40:80, :],
                        in_=xv[b * V + 45:(b + 1) * V, :].rearrange(
                            "c (p j) -> p c j", j=J
                        ),
                    )
                # score = cls * obj, in place, per class half. On DVE: pool
                # and DVE streaming SBUF concurrently starve each other
                # (measured 8-16x DVE slowdown), so a pool offload loses.
                for c0, c1 in ((0, 40), (40, 80)):
                    nc.vector.tensor_tensor(
                        out=xc[:, c0:c1, :],
                        in0=xc[:, c0:c1, :],
                        in1=xs[:, 4:5, :].broadcast_to((128, 40, J)),
                        op=op.mult,
                    )

                # centers/sizes on ACT (batched by function to avoid
                # activation-table reloads), grid add on DVE
                nc.scalar.activation(
                    out=bx6[:, 0:2, :], in_=xs[:, 0:2, :], func=Sigmoid
                )
                nc.scalar.activation(
                    out=bx6[:, 2:4, :], in_=xs[:, 2:4, :], func=Exp
                )
                nc.vector.tensor_add(out=bx6[:, 0:2, :], in0=bx6[:, 0:2, :], in1=gxy)
                nc.scalar.mul(
                    out=bx6[:, 2, :], in_=bx6[:, 2, :], mul=cg[:, 400 + t:401 + t]
                )
                nc.scalar.mul(
                    out=bx6[:, 3, :], in_=bx6[:, 3, :], mul=cg[:, 403 + t:404 + t]
                )

                # best = max over ch: pairwise max tree, all unit-stride
                # (DVE; the pool ucode rejects max)
                nc.vector.tensor_tensor(
                    out=scr, in0=xc[:, 0:40, :], in1=xc[:, 40:80, :], op=op.max
                )
                for w in (20, 10, 5):
                    nc.vector.tensor_tensor(
                        out=scr[:, 0:w, :], in0=scr[:, 0:w, :],
                        in1=scr[:, w:2 * w, :], op=op.max,
                    )
                nc.vector.tensor_tensor(
                    out=scr[:, 0:2, :], in0=scr[:, 0:2, :], in1=scr[:, 2:4, :],
                    op=op.max,
                )
                nc.vector.tensor_tensor(
                    out=scr[:, 0, :], in0=scr[:, 0, :], in1=scr[:, 1, :],
                    op=op.max,
                )
                nc.vector.tensor_tensor(
                    out=bx6[:, 4, :], in0=scr[:, 0, :], in1=scr[:, 4, :],
                    op=op.max,
                )

                # eq = (score == best) -> bf16, unit-stride
                nc.vector.tensor_tensor(
                    out=scrb,
                    in0=xc,
                    in1=bx6[:, 4:5, :].broadcast_to((128, CLS, J)),
                    op=op.is_equal,
                )
                # cand = eq * (ch - BIG), in place, all-bf16
                nc.vector.tensor_tensor(
                    out=scrb,
                    in0=scrb,
                    in1=iotb.unsqueeze(2).broadcast_to((128, CLS, J)),
                    op=op.mult,
                )
                # best_cls = min(cand) + BIG via bf16 pairwise min tree
                # (2x DVE mode; a tensor_reduce here measures 2x slower)
                nc.vector.tensor_tensor(
                    out=scrb[:, 0:40, :], in0=scrb[:, 0:40, :],
                    in1=scrb[:, 40:80, :], op=op.min,
                )
                for w in (20, 10, 5):
                    nc.vector.tensor_tensor(
                        out=scrb[:, 0:w, :], in0=scrb[:, 0:w, :],
                        in1=scrb[:, w:2 * w, :], op=op.min,
                    )
                nc.vector.tensor_tensor(
                    out=scrb[:, 0:2, :], in0=scrb[:, 0:2, :],
                    in1=scrb[:, 2:4, :], op=op.min,
                )
                nc.vector.tensor_tensor(
                    out=scrb[:, 0, :], in0=scrb[:, 0, :], in1=scrb[:, 1, :],
                    op=op.min,
                )
                cmin = outp.tile([128, J], bf16, tag="cmin")
                nc.vector.tensor_tensor(
                    out=cmin, in0=scrb[:, 0, :], in1=scrb[:, 4, :], op=op.min
                )
                nc.vector.tensor_scalar_add(
                    out=bx6[:, 5, :], in0=cmin, scalar1=BIG
                )

                for g in range(TPB):
                    b = t * TPB + g
                    dst = ov[b * 6:(b + 1) * 6, :].rearrange(
                        "k (p j) -> p k j", j=J
                    )
                    # SWDGE: the pool engine is otherwise idle, so output
                    # DMA dispatch never blocks a load queue or compute
                    nc.gpsimd.dma_start(
                        out=dst, in_=bx6[g * 32:(g + 1) * 32, :, :]
                    )

    return nc


def _assemble(core_outs):
    out = np.concatenate(core_outs, axis=0)           # [N, A, 6, HW]
    nb = out.shape[0]
    boxes = np.transpose(out, (0, 1, 3, 2)).reshape(nb, A, H, W, 6)
    boxes = np.ascontiguousarray(boxes, dtype=np.float32)
    mask = boxes[..., 4] > CONF_THR
    return boxes, mask


def kernel(**inputs):
    x = np.ascontiguousarray(inputs["x"], dtype=np.float32)
    anchors = np.asarray(inputs["anchors"], dtype=np.float32)
    assert x.shape == (N, C, H, W), x.shape

    from concourse.bass_utils import run_bass_kernel_spmd

    nc = _build(anchors)
    nc.finalize()  # Bacc lowering (reg alloc, wait splitting) before PJRT
    in_maps = [{"x": x[k * NSH:(k + 1) * NSH]} for k in range(NCORES)]
    res = run_bass_kernel_spmd(nc, in_maps, list(range(NCORES))).results
    return _assemble([res[k]["out"] for k in range(NCORES)])
